# revision 23
# baseline (speedup 1.0000x reference)
"""Trainium2 Bass kernel for ConvNdFunc: 16x16/stride-8 patch MLP (256->1024->1).

Data-parallel over batch: 32 images -> 8 cores x 4 images. Each core:
  - loads x rows with 8-fold kw-phase replication (partition = (kh, kw_phase),
    each phase shifted +1 column so stride-8 window reads become uniform APs)
  - L1: hT[hid128, 441win] += W1_chunk.T @ patchesT  (bf16, PSUM f32 accum,
    K=256 as 2 chunks; chunk1 = same buffer shifted one window column)
  - ScalarE ReLU -> bf16 SBUF
  - L2: M=1 matmuls with W2 chunk as stationary operand, PSUM-accumulated
  - + b2, store f32 windows
"""

import os
import sys
from contextlib import ExitStack

_RT = "/opt/trn_rl_repo"
if _RT not in sys.path:
    sys.path.insert(0, _RT)

import ml_dtypes
import numpy as np

def _ensure_ntff_hook():
    """Register the axon NTFF profiling hook if the image's antenv lacks it.

    Only matters when tracing (KERNEL_TRACE=1); no-op side effects otherwise.
    """
    import types

    try:
        import antenv.axon_hooks  # noqa: F401

        return
    except ImportError:
        pass
    try:
        import antenv
    except ImportError:
        return
    mod = types.ModuleType("antenv.axon_hooks")
    _state = {"hook": None}
    mod.set_axon_ntff_profile_hook = lambda h: _state.__setitem__("hook", h)
    mod.get_axon_ntff_profile_hook = lambda: _state["hook"]
    sys.modules["antenv.axon_hooks"] = mod
    antenv.axon_hooks = mod
    try:
        from trn_agent_boot.trn_boot import _ntff_profile_via_ctypes

        mod.set_axon_ntff_profile_hook(
            _ntff_profile_via_ctypes("/opt/axon/libaxon_pjrt.so")
        )
    except Exception:
        pass


_ensure_ntff_hook()

import concourse.bass as bass
import concourse.tile as tile
from concourse import bacc, mybir
from concourse.bass_utils import run_bass_kernel_spmd

B, H, W = 32, 512, 512
KK, S, HID = 16, 8, 1024
OH = OW = (H - KK) // S + 1  # 63
NCORES = 8
BPC = B // NCORES  # 4 images per core
WPAD = W + S  # pad columns so kw-phase-shifted row reads stay in bounds
G = 7  # window-rows per tile
NG = OH // G  # 9 tiles per image
NW = G * OW  # 441 real windows per tile
OWP = OW + 1  # padded window columns per row-group (j=63 is discarded)
NWP = G * OWP  # 448 matmul free dim per tile
NHB = HID // 128  # 8 hidden blocks

BF16 = ml_dtypes.bfloat16
F32 = mybir.dt.float32
BF16_T = mybir.dt.bfloat16

LAST_RESULTS = None  # BassKernelResults of the most recent run (for test harness)


def _build_nc(b2_val: float, b1_nonzero: bool):
    nc = bacc.Bacc(None, target_bir_lowering=False)

    x_d = nc.dram_tensor("x", [BPC, H, WPAD], BF16_T, kind="ExternalInput")
    w1_d = nc.dram_tensor("w1", [128, 2, HID], BF16_T, kind="ExternalInput")
    w2_d = nc.dram_tensor("w2", [128, NHB], BF16_T, kind="ExternalInput")
    b1_d = nc.dram_tensor("b1", [1, HID], BF16_T, kind="ExternalInput")
    y_d = nc.dram_tensor("y", [BPC, OH, OW], F32, kind="ExternalOutput")

    relu = mybir.ActivationFunctionType.Relu
    act_groups = [(0, 2), (2, 2), (4, 2), (6, 2)]  # (first hidden block, n blocks)

    with tile.TileContext(nc) as tc, ExitStack() as ctx:
        consts = ctx.enter_context(tc.tile_pool(name="consts", bufs=1))
        xr_pool = ctx.enter_context(tc.tile_pool(name="xr", bufs=3))
        yb_pool = ctx.enter_context(tc.tile_pool(name="yb", bufs=3))
        hs_pool = ctx.enter_context(tc.tile_pool(name="hs", bufs=3))
        osb_pool = ctx.enter_context(tc.tile_pool(name="osb", bufs=4))
        ht_pool = ctx.enter_context(tc.tile_pool(name="ht", bufs=6, space="PSUM"))
        ops_pool = ctx.enter_context(tc.tile_pool(name="ops", bufs=2, space="PSUM"))

        w1_sb = consts.tile([128, 2, HID], BF16_T)
        nc.sync.dma_start(out=w1_sb, in_=w1_d[:, :, :])
        w2_sb = consts.tile([128, NHB], BF16_T)
        nc.sync.dma_start(out=w2_sb, in_=w2_d[:, :])
        if b1_nonzero:
            b1_sb = consts.tile([1, HID], BF16_T)
            nc.sync.dma_start(out=b1_sb, in_=b1_d[:, :])
            ones_sb = consts.tile([1, NWP], BF16_T)
            nc.vector.memset(ones_sb, 1.0)

        for b in range(BPC):
            for g in range(NG):
                xr = xr_pool.tile([128, G, 512], BF16_T)
                # partition p = kh*8 + kwp holds x[b, 8*(G*g+i)+kh, kwp : kwp+512]
                # (DMA APs max 3 dims per side -> one DMA per window-row i)
                for i in range(G):
                    off = b * H * WPAD + (g * G + i) * S * WPAD
                    in_ap = bass.AP(
                        tensor=x_d.ap().tensor,
                        offset=off,
                        ap=[[WPAD, KK], [1, S], [1, 512]],
                    )
                    eng = nc.sync if i % 2 == 0 else nc.scalar
                    eng.dma_start(out=xr[:, i, :], in_=in_ap)

                # dense window buffer: yb[p, i*64 + j] = xr[p, i, 8*j]
                # (stride-8 DVE extract so matmul rhs is fully contiguous)
                yb = yb_pool.tile([128, NWP + S], BF16_T)
                in_ext = bass.AP(
                    tensor=xr.tensor,
                    offset=xr.offset,
                    ap=[xr.ap[0], [512, G], [S, OWP]],
                )
                nc.gpsimd.tensor_copy(yb[:, 0:NWP], in_ext)
                nc.gpsimd.memset(yb[:, NWP : NWP + S], 0.0)

                hs = hs_pool.tile([128, NHB, NWP], BF16_T)
                for hb in range(NHB):
                    ht = ht_pool.tile([128, 512], F32)
                    if b1_nonzero:
                        nc.tensor.matmul(
                            ht[:, 0:NWP],
                            b1_sb[:, hb * 128 : (hb + 1) * 128],
                            ones_sb[:, :],
                            start=True,
                            stop=False,
                        )
                    for c in range(2):
                        nc.tensor.matmul(
                            ht[:, 0:NWP],
                            w1_sb[:, c, hb * 128 : (hb + 1) * 128],
                            yb[:, c : c + NWP],
                            start=(c == 0 and not b1_nonzero),
                            stop=(c == 1),
                        )
                    if hb % 2 == 0:
                        nc.scalar.activation(
                            out=hs[:, hb, :], in_=ht[:, 0:NWP], func=relu
                        )
                    else:
                        nc.vector.tensor_scalar_max(
                            hs[:, hb, :], ht[:, 0:NWP], 0.0
                        )

                ops = ops_pool.tile([1, NWP], F32)
                for hb in range(NHB):
                    nc.tensor.matmul(
                        ops,
                        w2_sb[:, hb : hb + 1],
                        hs[:, hb, :],
                        start=(hb == 0),
                        stop=(hb == NHB - 1),
                    )

                osb = osb_pool.tile([1, NWP], F32)
                nc.vector.tensor_scalar_add(osb, ops, float(b2_val))
                out_src = bass.AP(
                    tensor=osb.tensor,
                    offset=osb.offset,
                    ap=[osb.ap[0], [OWP, G], [1, OW]],
                )
                nc.scalar.dma_start(out=y_d[b, g * G : (g + 1) * G, :], in_=out_src)

    nc.finalize()
    return nc


def kernel(x, W1, b1, W2, b2):
    global LAST_RESULTS
    x = np.asarray(x, dtype=np.float32)
    W1 = np.asarray(W1, dtype=np.float32)
    b1 = np.asarray(b1, dtype=np.float32)
    W2 = np.asarray(W2, dtype=np.float32)
    b2 = np.asarray(b2, dtype=np.float32)

    xp = np.zeros((B, H, WPAD), dtype=BF16)
    xp[:, :, :W] = x.astype(BF16)

    # W1 row r = kh*16 + kw; chunk c, partition p=(kh*8+kwp) <- row kh*16 + 8*c + kwp
    w1p = (
        W1.reshape(KK, 2, S, HID).transpose(0, 2, 1, 3).reshape(128, 2, HID)
    ).astype(BF16)
    w2p = W2.reshape(NHB, 128).T.copy().astype(BF16)  # [p, hb] = W2[hb*128+p]
    b1p = b1.reshape(1, HID).astype(BF16)
    b1_nonzero = bool(np.any(b1 != 0.0))
    b2_val = float(b2.reshape(-1)[0])

    nc = _build_nc(b2_val, b1_nonzero)

    in_maps = []
    for c in range(NCORES):
        in_maps.append(
            {
                "x": np.ascontiguousarray(xp[c * BPC : (c + 1) * BPC]),
                "w1": w1p,
                "w2": w2p,
                "b1": b1p,
            }
        )

    LAST_RESULTS = run_bass_kernel_spmd(
        nc,
        in_maps,
        core_ids=list(range(NCORES)),
        trace=bool(int(os.environ.get("KERNEL_TRACE", "0") or "0")),
    )
    y = np.concatenate([r["y"] for r in LAST_RESULTS.results], axis=0)
    return y.astype(np.float32)


# revision 25
# speedup vs baseline: 1.0667x; 1.0667x over previous
"""Trainium2 Bass kernel for ConvNdFunc: 16x16/stride-8 patch MLP (256->1024->1).

Data-parallel over batch: 32 images -> 8 cores x 4 images. Each core:
  - loads x rows with 8-fold kw-phase replication (partition = (kh, kw_phase),
    each phase shifted +1 column so stride-8 window reads become uniform APs)
  - L1: hT[hid128, 441win] += W1_chunk.T @ patchesT  (bf16, PSUM f32 accum,
    K=256 as 2 chunks; chunk1 = same buffer shifted one window column)
  - ScalarE ReLU -> bf16 SBUF
  - L2: M=1 matmuls with W2 chunk as stationary operand, PSUM-accumulated
  - + b2, store f32 windows
"""

import os
import sys
from contextlib import ExitStack

_RT = "/opt/trn_rl_repo"
if _RT not in sys.path:
    sys.path.insert(0, _RT)

import ml_dtypes
import numpy as np

def _ensure_ntff_hook():
    """Register the axon NTFF profiling hook if the image's antenv lacks it.

    Only matters when tracing (KERNEL_TRACE=1); no-op side effects otherwise.
    """
    import types

    try:
        import antenv.axon_hooks  # noqa: F401

        return
    except ImportError:
        pass
    try:
        import antenv
    except ImportError:
        return
    mod = types.ModuleType("antenv.axon_hooks")
    _state = {"hook": None}
    mod.set_axon_ntff_profile_hook = lambda h: _state.__setitem__("hook", h)
    mod.get_axon_ntff_profile_hook = lambda: _state["hook"]
    sys.modules["antenv.axon_hooks"] = mod
    antenv.axon_hooks = mod
    try:
        from trn_agent_boot.trn_boot import _ntff_profile_via_ctypes

        mod.set_axon_ntff_profile_hook(
            _ntff_profile_via_ctypes("/opt/axon/libaxon_pjrt.so")
        )
    except Exception:
        pass


_ensure_ntff_hook()

import concourse.bass as bass
import concourse.tile as tile
from concourse import bacc, mybir
from concourse.bass_utils import run_bass_kernel_spmd

B, H, W = 32, 512, 512
KK, S, HID = 16, 8, 1024
OH = OW = (H - KK) // S + 1  # 63
NCORES = 8
BPC = B // NCORES  # 4 images per core
WPAD = W + S  # pad columns so kw-phase-shifted row reads stay in bounds
G = 7  # window-rows per tile
NG = OH // G  # 9 tiles per image
NW = G * OW  # 441 real windows per tile
OWP = OW + 1  # padded window columns per row-group (j=63 is discarded)
NWP = G * OWP  # 448 matmul free dim per tile
NHB = HID // 128  # 8 hidden blocks

BF16 = ml_dtypes.bfloat16
F32 = mybir.dt.float32
BF16_T = mybir.dt.bfloat16

LAST_RESULTS = None  # BassKernelResults of the most recent run (for test harness)


def _build_nc(b2_val: float, b1_nonzero: bool):
    nc = bacc.Bacc(None, target_bir_lowering=False)

    x_d = nc.dram_tensor("x", [BPC, H, WPAD], BF16_T, kind="ExternalInput")
    w1_d = nc.dram_tensor("w1", [128, 2, HID], BF16_T, kind="ExternalInput")
    w2_d = nc.dram_tensor("w2", [128, NHB], BF16_T, kind="ExternalInput")
    b1_d = nc.dram_tensor("b1", [1, HID], BF16_T, kind="ExternalInput")
    y_d = nc.dram_tensor("y", [BPC, OH, OW], F32, kind="ExternalOutput")

    relu = mybir.ActivationFunctionType.Relu
    act_groups = [(0, 2), (2, 2), (4, 2), (6, 2)]  # (first hidden block, n blocks)

    with tile.TileContext(nc) as tc, ExitStack() as ctx:
        consts = ctx.enter_context(tc.tile_pool(name="consts", bufs=1))
        xr_pool = ctx.enter_context(tc.tile_pool(name="xr", bufs=3))
        yb_pool = ctx.enter_context(tc.tile_pool(name="yb", bufs=3))
        hs_pool = ctx.enter_context(tc.tile_pool(name="hs", bufs=3))
        osb_pool = ctx.enter_context(tc.tile_pool(name="osb", bufs=4))
        ht_pool = ctx.enter_context(tc.tile_pool(name="ht", bufs=6, space="PSUM"))
        ops_pool = ctx.enter_context(tc.tile_pool(name="ops", bufs=2, space="PSUM"))

        w1_sb = consts.tile([128, 2, HID], BF16_T)
        nc.scalar.dma_start(out=w1_sb, in_=w1_d[:, :, :])
        w2_sb = consts.tile([128, NHB], BF16_T)
        nc.scalar.dma_start(out=w2_sb, in_=w2_d[:, :])
        if b1_nonzero:
            b1_sb = consts.tile([1, HID], BF16_T)
            nc.sync.dma_start(out=b1_sb, in_=b1_d[:, :])
            ones_sb = consts.tile([1, NWP], BF16_T)
            nc.vector.memset(ones_sb, 1.0)

        for b in range(BPC):
            for g in range(NG):
                xr = xr_pool.tile([128, G, 512], BF16_T)
                # partition p = kh*8 + kwp holds x[b, 8*(G*g+i)+kh, kwp : kwp+512]
                # (DMA APs max 3 dims per side -> one DMA per window-row i)
                for i in range(G):
                    off = b * H * WPAD + (g * G + i) * S * WPAD
                    in_ap = bass.AP(
                        tensor=x_d.ap().tensor,
                        offset=off,
                        ap=[[WPAD, KK], [1, S], [1, 512]],
                    )
                    nc.sync.dma_start(out=xr[:, i, :], in_=in_ap)

                # dense window buffer: yb[p, i*64 + j] = xr[p, i, 8*j]
                # (stride-8 DVE extract so matmul rhs is fully contiguous)
                yb = yb_pool.tile([128, NWP + S], BF16_T)
                in_ext = bass.AP(
                    tensor=xr.tensor,
                    offset=xr.offset,
                    ap=[xr.ap[0], [512, G], [S, OWP]],
                )
                nc.gpsimd.tensor_copy(yb[:, 0:NWP], in_ext)
                nc.gpsimd.memset(yb[:, NWP : NWP + S], 0.0)

                hs = hs_pool.tile([128, NHB, NWP], BF16_T)
                for hb in range(NHB):
                    ht = ht_pool.tile([128, 512], F32)
                    if b1_nonzero:
                        nc.tensor.matmul(
                            ht[:, 0:NWP],
                            b1_sb[:, hb * 128 : (hb + 1) * 128],
                            ones_sb[:, :],
                            start=True,
                            stop=False,
                        )
                    for c in range(2):
                        nc.tensor.matmul(
                            ht[:, 0:NWP],
                            w1_sb[:, c, hb * 128 : (hb + 1) * 128],
                            yb[:, c : c + NWP],
                            start=(c == 0 and not b1_nonzero),
                            stop=(c == 1),
                        )
                    if hb in (0, 2, 4):
                        nc.scalar.activation(
                            out=hs[:, hb, :], in_=ht[:, 0:NWP], func=relu
                        )
                    else:
                        nc.vector.tensor_scalar_max(
                            hs[:, hb, :], ht[:, 0:NWP], 0.0
                        )

                ops = ops_pool.tile([1, NWP], F32)
                for hb in range(NHB):
                    nc.tensor.matmul(
                        ops,
                        w2_sb[:, hb : hb + 1],
                        hs[:, hb, :],
                        start=(hb == 0),
                        stop=(hb == NHB - 1),
                    )

                osb = osb_pool.tile([1, NWP], F32)
                nc.scalar.activation(
                    out=osb,
                    in_=ops,
                    func=mybir.ActivationFunctionType.Copy,
                    bias=float(b2_val),
                )
                out_src = bass.AP(
                    tensor=osb.tensor,
                    offset=osb.offset,
                    ap=[osb.ap[0], [OWP, G], [1, OW]],
                )
                nc.scalar.dma_start(out=y_d[b, g * G : (g + 1) * G, :], in_=out_src)

    nc.finalize()
    return nc


def kernel(x, W1, b1, W2, b2):
    global LAST_RESULTS
    x = np.asarray(x, dtype=np.float32)
    W1 = np.asarray(W1, dtype=np.float32)
    b1 = np.asarray(b1, dtype=np.float32)
    W2 = np.asarray(W2, dtype=np.float32)
    b2 = np.asarray(b2, dtype=np.float32)

    xp = np.zeros((B, H, WPAD), dtype=BF16)
    xp[:, :, :W] = x.astype(BF16)

    # W1 row r = kh*16 + kw; chunk c, partition p=(kh*8+kwp) <- row kh*16 + 8*c + kwp
    w1p = (
        W1.reshape(KK, 2, S, HID).transpose(0, 2, 1, 3).reshape(128, 2, HID)
    ).astype(BF16)
    w2p = W2.reshape(NHB, 128).T.copy().astype(BF16)  # [p, hb] = W2[hb*128+p]
    b1p = b1.reshape(1, HID).astype(BF16)
    b1_nonzero = bool(np.any(b1 != 0.0))
    b2_val = float(b2.reshape(-1)[0])

    nc = _build_nc(b2_val, b1_nonzero)

    in_maps = []
    for c in range(NCORES):
        in_maps.append(
            {
                "x": np.ascontiguousarray(xp[c * BPC : (c + 1) * BPC]),
                "w1": w1p,
                "w2": w2p,
                "b1": b1p,
            }
        )

    LAST_RESULTS = run_bass_kernel_spmd(
        nc,
        in_maps,
        core_ids=list(range(NCORES)),
        trace=bool(int(os.environ.get("KERNEL_TRACE", "0") or "0")),
    )
    y = np.concatenate([r["y"] for r in LAST_RESULTS.results], axis=0)
    return y.astype(np.float32)


# revision 26
# speedup vs baseline: 1.0722x; 1.0051x over previous
"""Trainium2 Bass kernel for ConvNdFunc: 16x16/stride-8 patch MLP (256->1024->1).

Data-parallel over batch: 32 images -> 8 cores x 4 images. Each core:
  - loads x rows with 8-fold kw-phase replication (partition = (kh, kw_phase),
    each phase shifted +1 column so stride-8 window reads become uniform APs)
  - L1: hT[hid128, 441win] += W1_chunk.T @ patchesT  (bf16, PSUM f32 accum,
    K=256 as 2 chunks; chunk1 = same buffer shifted one window column)
  - ScalarE ReLU -> bf16 SBUF
  - L2: M=1 matmuls with W2 chunk as stationary operand, PSUM-accumulated
  - + b2, store f32 windows
"""

import os
import sys
from contextlib import ExitStack

_RT = "/opt/trn_rl_repo"
if _RT not in sys.path:
    sys.path.insert(0, _RT)

import ml_dtypes
import numpy as np

def _ensure_ntff_hook():
    """Register the axon NTFF profiling hook if the image's antenv lacks it.

    Only matters when tracing (KERNEL_TRACE=1); no-op side effects otherwise.
    """
    import types

    try:
        import antenv.axon_hooks  # noqa: F401

        return
    except ImportError:
        pass
    try:
        import antenv
    except ImportError:
        return
    mod = types.ModuleType("antenv.axon_hooks")
    _state = {"hook": None}
    mod.set_axon_ntff_profile_hook = lambda h: _state.__setitem__("hook", h)
    mod.get_axon_ntff_profile_hook = lambda: _state["hook"]
    sys.modules["antenv.axon_hooks"] = mod
    antenv.axon_hooks = mod
    try:
        from trn_agent_boot.trn_boot import _ntff_profile_via_ctypes

        mod.set_axon_ntff_profile_hook(
            _ntff_profile_via_ctypes("/opt/axon/libaxon_pjrt.so")
        )
    except Exception:
        pass


_ensure_ntff_hook()

import concourse.bass as bass
import concourse.tile as tile
from concourse import bacc, mybir
from concourse.bass_utils import run_bass_kernel_spmd

B, H, W = 32, 512, 512
KK, S, HID = 16, 8, 1024
OH = OW = (H - KK) // S + 1  # 63
NCORES = 8
BPC = B // NCORES  # 4 images per core
WPAD = W + S  # pad columns so kw-phase-shifted row reads stay in bounds
G = 7  # window-rows per tile
NG = OH // G  # 9 tiles per image
NW = G * OW  # 441 real windows per tile
OWP = OW + 1  # padded window columns per row-group (j=63 is discarded)
NWP = G * OWP  # 448 matmul free dim per tile
NHB = HID // 128  # 8 hidden blocks

BF16 = ml_dtypes.bfloat16
F32 = mybir.dt.float32
BF16_T = mybir.dt.bfloat16

LAST_RESULTS = None  # BassKernelResults of the most recent run (for test harness)


def _build_nc(b2_val: float, b1_nonzero: bool):
    nc = bacc.Bacc(None, target_bir_lowering=False)

    x_d = nc.dram_tensor("x", [BPC, H, WPAD], BF16_T, kind="ExternalInput")
    w1_d = nc.dram_tensor("w1", [128, 2, HID], BF16_T, kind="ExternalInput")
    w2_d = nc.dram_tensor("w2", [128, NHB], BF16_T, kind="ExternalInput")
    b1_d = nc.dram_tensor("b1", [1, HID], BF16_T, kind="ExternalInput")
    y_d = nc.dram_tensor("y", [BPC, OH, OW], F32, kind="ExternalOutput")

    relu = mybir.ActivationFunctionType.Relu
    act_groups = [(0, 2), (2, 2), (4, 2), (6, 2)]  # (first hidden block, n blocks)

    with tile.TileContext(nc) as tc, ExitStack() as ctx:
        consts = ctx.enter_context(tc.tile_pool(name="consts", bufs=1))
        xr_pool = ctx.enter_context(tc.tile_pool(name="xr", bufs=3))
        yb_pool = ctx.enter_context(tc.tile_pool(name="yb", bufs=3))
        hs_pool = ctx.enter_context(tc.tile_pool(name="hs", bufs=3))
        osb_pool = ctx.enter_context(tc.tile_pool(name="osb", bufs=4))
        ht_pool = ctx.enter_context(tc.tile_pool(name="ht", bufs=6, space="PSUM"))
        ops_pool = ctx.enter_context(tc.tile_pool(name="ops", bufs=2, space="PSUM"))

        w1_sb = consts.tile([128, 2, HID], BF16_T)
        nc.scalar.dma_start(out=w1_sb, in_=w1_d[:, :, :])
        w2_sb = consts.tile([128, NHB], BF16_T)
        nc.scalar.dma_start(out=w2_sb, in_=w2_d[:, :])
        if b1_nonzero:
            b1_sb = consts.tile([1, HID], BF16_T)
            nc.sync.dma_start(out=b1_sb, in_=b1_d[:, :])
            ones_sb = consts.tile([1, NWP], BF16_T)
            nc.vector.memset(ones_sb, 1.0)

        pending = []

        def _drain_output(item):
            ops, bb, gg = item
            osb = osb_pool.tile([1, NWP], F32)
            nc.scalar.activation(
                out=osb,
                in_=ops,
                func=mybir.ActivationFunctionType.Copy,
                bias=float(b2_val),
            )
            out_src = bass.AP(
                tensor=osb.tensor,
                offset=osb.offset,
                ap=[osb.ap[0], [OWP, G], [1, OW]],
            )
            nc.scalar.dma_start(
                out=y_d[bb, gg * G : (gg + 1) * G, :], in_=out_src
            )

        for b in range(BPC):
            for g in range(NG):
                xr = xr_pool.tile([128, G, 512], BF16_T)
                # partition p = kh*8 + kwp holds x[b, 8*(G*g+i)+kh, kwp : kwp+512]
                # (DMA APs max 3 dims per side -> one DMA per window-row i)
                for i in range(G):
                    off = b * H * WPAD + (g * G + i) * S * WPAD
                    in_ap = bass.AP(
                        tensor=x_d.ap().tensor,
                        offset=off,
                        ap=[[WPAD, KK], [1, S], [1, 512]],
                    )
                    nc.sync.dma_start(out=xr[:, i, :], in_=in_ap)

                # dense window buffer: yb[p, i*64 + j] = xr[p, i, 8*j]
                # (stride-8 DVE extract so matmul rhs is fully contiguous)
                yb = yb_pool.tile([128, NWP + S], BF16_T)
                in_ext = bass.AP(
                    tensor=xr.tensor,
                    offset=xr.offset,
                    ap=[xr.ap[0], [512, G], [S, OWP]],
                )
                nc.gpsimd.tensor_copy(yb[:, 0:NWP], in_ext)
                nc.gpsimd.memset(yb[:, NWP : NWP + S], 0.0)

                hs = hs_pool.tile([128, NHB, NWP], BF16_T)
                for hb in range(NHB):
                    ht = ht_pool.tile([128, 512], F32)
                    if b1_nonzero:
                        nc.tensor.matmul(
                            ht[:, 0:NWP],
                            b1_sb[:, hb * 128 : (hb + 1) * 128],
                            ones_sb[:, :],
                            start=True,
                            stop=False,
                        )
                    for c in range(2):
                        nc.tensor.matmul(
                            ht[:, 0:NWP],
                            w1_sb[:, c, hb * 128 : (hb + 1) * 128],
                            yb[:, c : c + NWP],
                            start=(c == 0 and not b1_nonzero),
                            stop=(c == 1),
                        )
                    if hb in (0, 2, 4):
                        nc.scalar.activation(
                            out=hs[:, hb, :], in_=ht[:, 0:NWP], func=relu
                        )
                    else:
                        nc.vector.tensor_scalar_max(
                            hs[:, hb, :], ht[:, 0:NWP], 0.0
                        )

                ops = ops_pool.tile([1, NWP], F32)
                for hb in range(NHB):
                    nc.tensor.matmul(
                        ops,
                        w2_sb[:, hb : hb + 1],
                        hs[:, hb, :],
                        start=(hb == 0),
                        stop=(hb == NHB - 1),
                    )

                # defer the output chain by one tile so its FIFO entries never
                # block the next tile's relu ops with an unsatisfied wait
                pending.append((ops, b, g))
                if len(pending) > 1:
                    _drain_output(pending.pop(0))

        while pending:
            _drain_output(pending.pop(0))

    nc.finalize()
    return nc


def kernel(x, W1, b1, W2, b2):
    global LAST_RESULTS
    x = np.asarray(x, dtype=np.float32)
    W1 = np.asarray(W1, dtype=np.float32)
    b1 = np.asarray(b1, dtype=np.float32)
    W2 = np.asarray(W2, dtype=np.float32)
    b2 = np.asarray(b2, dtype=np.float32)

    xp = np.zeros((B, H, WPAD), dtype=BF16)
    xp[:, :, :W] = x.astype(BF16)

    # W1 row r = kh*16 + kw; chunk c, partition p=(kh*8+kwp) <- row kh*16 + 8*c + kwp
    w1p = (
        W1.reshape(KK, 2, S, HID).transpose(0, 2, 1, 3).reshape(128, 2, HID)
    ).astype(BF16)
    w2p = W2.reshape(NHB, 128).T.copy().astype(BF16)  # [p, hb] = W2[hb*128+p]
    b1p = b1.reshape(1, HID).astype(BF16)
    b1_nonzero = bool(np.any(b1 != 0.0))
    b2_val = float(b2.reshape(-1)[0])

    nc = _build_nc(b2_val, b1_nonzero)

    in_maps = []
    for c in range(NCORES):
        in_maps.append(
            {
                "x": np.ascontiguousarray(xp[c * BPC : (c + 1) * BPC]),
                "w1": w1p,
                "w2": w2p,
                "b1": b1p,
            }
        )

    LAST_RESULTS = run_bass_kernel_spmd(
        nc,
        in_maps,
        core_ids=list(range(NCORES)),
        trace=bool(int(os.environ.get("KERNEL_TRACE", "0") or "0")),
    )
    y = np.concatenate([r["y"] for r in LAST_RESULTS.results], axis=0)
    return y.astype(np.float32)
